# revision 12
# baseline (speedup 1.0000x reference)
"""Trainium2 Bass kernel for nn_MixedActivation.

Column i of x uses activation (i % 6): 0,1,2 -> square; 3,4,5 -> PReLU with
prelu_a[0..2]. Data-parallel over rows across 8 NeuronCores.

Layout: per core shard [125000, 48] f32, processed as tiles where partition p
holds B=50 consecutive rows (48*B = 2400 contiguous floats in DRAM). The
mod-6 column pattern becomes a period-6 pattern along the free dim, so each
column class is covered by one strided-AP instruction per tile:
  - squares (phases 0,1,2): DVE tensor_tensor mult, AP [[48,B],[6,8],[1,3]]
  - prelu k (phase 3+k):    ACT Prelu with immediate alpha, AP [[48,B],[6,8]]
In-DMAs issue on SP's HWDGE queue, out-DMAs on ACT's HWDGE queue.
"""

import numpy as np

import concourse.bass as bass
import concourse.mybir as mybir
from concourse.bass_utils import run_bass_kernel_spmd

N_CORES = 8
ROWS = 1_000_000
COLS = 48
SHARD_ROWS = ROWS // N_CORES  # 125000

B = 100                 # rows per partition per tile
P = 128                 # partitions
TILE_ROWS = P * B       # 12800
N_FULL = SHARD_ROWS // TILE_ROWS          # 9
TAIL_ROWS = SHARD_ROWS - N_FULL * TILE_ROWS  # 9800
TAIL_P = TAIL_ROWS // B                   # 98
NTILES = N_FULL + 1
NB = 4                  # buffers
F = COLS * B            # 4800 floats per partition


def _build(prelu_a, replicas=1, B=B, NB=NB):
    """Build the per-core BIR program.

    replicas>1 unrolls the whole pipeline K times over the same data —
    used only for timing (K-replica differencing isolates HW exec time
    from host/dispatch overhead).
    """
    TILE_ROWS = P * B
    N_FULL = SHARD_ROWS // TILE_ROWS
    TAIL_ROWS = SHARD_ROWS - N_FULL * TILE_ROWS
    assert TAIL_ROWS % B == 0
    TAIL_P = TAIL_ROWS // B
    NTILES = N_FULL + (1 if TAIL_ROWS else 0)
    F = COLS * B
    a0, a1, a2 = (float(v) for v in prelu_a)
    nc = bass.Bass("TRN2", target_bir_lowering=False)
    x_ext = nc.declare_dram_parameter(
        "x", [SHARD_ROWS, COLS], mybir.dt.float32, isOutput=False
    )
    y_ext = nc.declare_dram_parameter(
        "y", [SHARD_ROWS, COLS], mybir.dt.float32, isOutput=True
    )

    # DRAM tile views: [n, p, b*c] with contiguous per-partition chunks
    x_full = x_ext[0 : N_FULL * TILE_ROWS, :].rearrange(
        "(n p b) c -> n p (b c)", n=N_FULL, p=P, b=B
    )
    y_full = y_ext[0 : N_FULL * TILE_ROWS, :].rearrange(
        "(n p b) c -> n p (b c)", n=N_FULL, p=P, b=B
    )
    x_tail = x_ext[N_FULL * TILE_ROWS :, :].rearrange(
        "(p b) c -> p (b c)", p=TAIL_P, b=B
    )
    y_tail = y_ext[N_FULL * TILE_ROWS :, :].rearrange(
        "(p b) c -> p (b c)", p=TAIL_P, b=B
    )

    def dram_in(i):
        return x_full[i] if i < N_FULL else x_tail

    def dram_out(i):
        return y_full[i] if i < N_FULL else y_tail

    def pdim(i):
        return P if i < N_FULL else TAIL_P

    from contextlib import ExitStack

    with ExitStack() as stack:
        tin = stack.enter_context(nc.sbuf_tensor([P, NB * F], mybir.dt.float32))
        tout = stack.enter_context(nc.sbuf_tensor([P, NB * F], mybir.dt.float32))
        # Per-buffer-slot DMA sems: a single counting sem shared by many
        # in-flight DMAs is racy — HWDGE fans one DMA across several HW
        # queues whose portions complete out of order ACROSS DMAs, so
        # sem >= 16*(t+1) can fire while DMA t is still partially in
        # flight. With one sem per buffer slot, consecutive DMAs on the
        # same sem are serialized by the pipeline's own dependency chain
        # (load t+NB is only issued after compute t, which needed load t
        # complete), so counts are exact.
        in_sems = [
            stack.enter_context(nc.semaphore(f"in_sem{b}")) for b in range(NB)
        ]
        out_sems = [
            stack.enter_context(nc.semaphore(f"out_sem{b}")) for b in range(NB)
        ]
        sq_sem = stack.enter_context(nc.semaphore("sq_sem"))
        pr_sem = stack.enter_context(nc.semaphore("pr_sem"))
        block = stack.enter_context(nc.Block())
        def in_buf(t):
            i = t % NTILES
            return tin[: pdim(i), (t % NB) * F : (t % NB + 1) * F]

        def out_buf(t):
            i = t % NTILES
            return tout[: pdim(i), (t % NB) * F : (t % NB + 1) * F]

        NT = NTILES * replicas

        def dti(t):  # schedule index -> dram tile index
            return t % NTILES

        def n_loads(t):  # value of in_sems[t % NB] after load of tile t
            return 16 * (t // NB + 1)

        @block.sync
        def _(sync):
            for t in range(min(NB, NT)):
                sync.dma_start(out=in_buf(t), in_=dram_in(dti(t))).then_inc(
                    in_sems[t % NB], 16
                )
            for t in range(NT):
                # issue next load once compute for tile t has consumed tin buf
                if t + NB < NT:
                    sync.wait_ge(sq_sem, t + 1)
                    sync.wait_ge(pr_sem, t + 1)
                    sync.dma_start(
                        out=in_buf(t + NB), in_=dram_in(dti(t + NB))
                    ).then_inc(in_sems[t % NB], 16)

        @block.scalar
        def _(scalar):
            for t in range(NT):
                i = dti(t)
                scalar.wait_ge(in_sems[t % NB], n_loads(t))
                if t >= NB:
                    # out-DMA of tile t-NB (same buffer slot) must be done
                    scalar.wait_ge(out_sems[t % NB], n_loads(t - NB))
                vin = in_buf(t).rearrange("p (b g s) -> p b g s", b=B, g=8, s=6)
                vout = out_buf(t).rearrange("p (b g s) -> p b g s", b=B, g=8, s=6)
                for k, a in enumerate((a0, a1, a2)):
                    scalar.activation(
                        out=vout[:, :, :, 3 + k : 4 + k],
                        in_=vin[:, :, :, 3 + k : 4 + k],
                        func=mybir.ActivationFunctionType.Prelu,
                        alpha=a,
                    )
                # drain flushes ACT's SBUF writes before the sem inc fires
                scalar.drain().then_inc(pr_sem, 1)
                # out-DMA issued here on ACT's HWDGE queue (separate from SP's)
                scalar.wait_ge(sq_sem, t + 1)
                scalar.dma_start(out=dram_out(i), in_=out_buf(t)).then_inc(
                    out_sems[t % NB], 16
                )
            for b in range(min(NB, NT)):
                last_t = NT - 1 - (NT - 1 - b) % NB  # last schedule slot on b
                scalar.wait_ge(out_sems[b], n_loads(last_t))

        @block.vector
        def _(vector):
            for t in range(NT):
                vector.wait_ge(in_sems[t % NB], n_loads(t))
                if t >= NB:
                    vector.wait_ge(out_sems[t % NB], n_loads(t - NB))
                vin = in_buf(t).rearrange("p (b g s) -> p b g s", b=B, g=8, s=6)
                vout = out_buf(t).rearrange("p (b g s) -> p b g s", b=B, g=8, s=6)
                vector.tensor_tensor(
                    out=vout[:, :, :, 0:3],
                    in0=vin[:, :, :, 0:3],
                    in1=vin[:, :, :, 0:3],
                    op=mybir.AluOpType.mult,
                )
                vector.drain().then_inc(sq_sem, 1)

    return nc


def kernel(x: np.ndarray, prelu_a: np.ndarray, trace: bool = False):
    nc = _build(prelu_a)
    x = np.ascontiguousarray(x, dtype=np.float32)
    in_maps = [
        {"x": x[c * SHARD_ROWS : (c + 1) * SHARD_ROWS]} for c in range(N_CORES)
    ]
    res = run_bass_kernel_spmd(nc, in_maps, list(range(N_CORES)), trace=trace)
    out = np.concatenate([res.results[c]["y"] for c in range(N_CORES)], axis=0)
    if trace:
        return out, res
    return out


# revision 13
# speedup vs baseline: 1.1434x; 1.1434x over previous
"""Trainium2 Bass kernel for nn_MixedActivation.

Column i of x uses activation (i % 6): 0,1,2 -> square; 3,4,5 -> PReLU with
prelu_a[0..2]. Data-parallel over rows across 8 NeuronCores.

Layout: per core shard [125000, 48] f32, processed as tiles where partition p
holds B=50 consecutive rows (48*B = 2400 contiguous floats in DRAM). The
mod-6 column pattern becomes a period-6 pattern along the free dim, so each
column class is covered by one strided-AP instruction per tile:
  - squares (phases 0,1,2): DVE tensor_tensor mult, AP [[48,B],[6,8],[1,3]]
  - prelu k (phase 3+k):    ACT Prelu with immediate alpha, AP [[48,B],[6,8]]
In-DMAs issue on SP's HWDGE queue, out-DMAs on ACT's HWDGE queue.
"""

import numpy as np

import concourse.bass as bass
import concourse.mybir as mybir
from concourse.bass_utils import run_bass_kernel_spmd

N_CORES = 8
ROWS = 1_000_000
COLS = 48
SHARD_ROWS = ROWS // N_CORES  # 125000

B = 100                 # rows per partition per tile
P = 128                 # partitions
TILE_ROWS = P * B       # 12800
N_FULL = SHARD_ROWS // TILE_ROWS          # 9
TAIL_ROWS = SHARD_ROWS - N_FULL * TILE_ROWS  # 9800
TAIL_P = TAIL_ROWS // B                   # 98
NTILES = N_FULL + 1
NB = 8                  # buffer slots (single in-place buffer array)
F = COLS * B            # 4800 floats per partition


def _build(prelu_a, replicas=1, B=B, NB=NB):
    """Build the per-core BIR program (in-place compute, single buffer).

    Compute happens in place in the input tile: DVE squares its strided
    view, ACT applies Prelu to its views, and the out-DMA reads the same
    buffer. Single buffer array + NB=8 slots gives deep DMA pipelining.
    Per-slot sems keep DMA completion counts exact (HWDGE multi-queue
    fan-out makes shared counting sems racy). Load t+NB is gated on the
    out-DMA of tile t having fully read the slot (WAR).

    replicas>1 unrolls the whole pipeline K times over the same data -
    used only for timing (K-replica differencing isolates HW exec time
    from host/dispatch overhead).
    """
    TILE_ROWS = P * B
    N_FULL = SHARD_ROWS // TILE_ROWS
    TAIL_ROWS = SHARD_ROWS - N_FULL * TILE_ROWS
    assert TAIL_ROWS % B == 0
    TAIL_P = TAIL_ROWS // B
    NTILES = N_FULL + (1 if TAIL_ROWS else 0)
    F = COLS * B
    a0, a1, a2 = (float(v) for v in prelu_a)
    nc = bass.Bass("TRN2", target_bir_lowering=False)
    x_ext = nc.declare_dram_parameter(
        "x", [SHARD_ROWS, COLS], mybir.dt.float32, isOutput=False
    )
    y_ext = nc.declare_dram_parameter(
        "y", [SHARD_ROWS, COLS], mybir.dt.float32, isOutput=True
    )

    # DRAM tile views: [n, p, b*c] with contiguous per-partition chunks
    x_full = x_ext[0 : N_FULL * TILE_ROWS, :].rearrange(
        "(n p b) c -> n p (b c)", n=N_FULL, p=P, b=B
    )
    y_full = y_ext[0 : N_FULL * TILE_ROWS, :].rearrange(
        "(n p b) c -> n p (b c)", n=N_FULL, p=P, b=B
    )
    x_tail = x_ext[N_FULL * TILE_ROWS :, :].rearrange(
        "(p b) c -> p (b c)", p=TAIL_P, b=B
    )
    y_tail = y_ext[N_FULL * TILE_ROWS :, :].rearrange(
        "(p b) c -> p (b c)", p=TAIL_P, b=B
    )

    def dram_in(i):
        return x_full[i] if i < N_FULL else x_tail

    def dram_out(i):
        return y_full[i] if i < N_FULL else y_tail

    def pdim(i):
        return P if i < N_FULL else TAIL_P

    from contextlib import ExitStack

    with ExitStack() as stack:
        tin = stack.enter_context(nc.sbuf_tensor([P, NB * F], mybir.dt.float32))
        in_sems = [
            stack.enter_context(nc.semaphore(f"in_sem{b}")) for b in range(NB)
        ]
        out_sems = [
            stack.enter_context(nc.semaphore(f"out_sem{b}")) for b in range(NB)
        ]
        sq_sem = stack.enter_context(nc.semaphore("sq_sem"))
        pr_sem = stack.enter_context(nc.semaphore("pr_sem"))
        block = stack.enter_context(nc.Block())

        NT = NTILES * replicas

        def dti(t):  # schedule index -> dram tile index
            return t % NTILES

        def buf(t):
            return tin[: pdim(dti(t)), (t % NB) * F : (t % NB + 1) * F]

        def n_loads(t):  # value of in_sems[t % NB] after load of tile t
            return 16 * (t // NB + 1)

        @block.sync
        def _(sync):
            for t in range(min(NB, NT)):
                sync.dma_start(out=buf(t), in_=dram_in(dti(t))).then_inc(
                    in_sems[t % NB], 16
                )
            for t in range(NT):
                if t + NB < NT:
                    # WAR: reload slot only after out-DMA t fully read it
                    sync.wait_ge(out_sems[t % NB], n_loads(t))
                    sync.dma_start(
                        out=buf(t + NB), in_=dram_in(dti(t + NB))
                    ).then_inc(in_sems[t % NB], 16)

        @block.scalar
        def _(scalar):
            for t in range(NT):
                i = dti(t)
                scalar.wait_ge(in_sems[t % NB], n_loads(t))
                v = buf(t).rearrange("p (b g s) -> p b g s", b=B, g=8, s=6)
                for k, a in enumerate((a0, a1, a2)):
                    scalar.activation(
                        out=v[:, :, :, 3 + k : 4 + k],
                        in_=v[:, :, :, 3 + k : 4 + k],
                        func=mybir.ActivationFunctionType.Prelu,
                        alpha=a,
                    )
                # drain flushes ACT's SBUF writes before the sem inc fires
                scalar.drain().then_inc(pr_sem, 1)
                # out-DMA on ACT's HWDGE queue (separate from SP's)
                scalar.wait_ge(sq_sem, t + 1)
                scalar.dma_start(out=dram_out(i), in_=buf(t)).then_inc(
                    out_sems[t % NB], 16
                )
            for b in range(min(NB, NT)):
                last_t = NT - 1 - (NT - 1 - b) % NB  # last schedule slot on b
                scalar.wait_ge(out_sems[b], n_loads(last_t))

        @block.vector
        def _(vector):
            for t in range(NT):
                vector.wait_ge(in_sems[t % NB], n_loads(t))
                v = buf(t).rearrange("p (b g s) -> p b g s", b=B, g=8, s=6)
                vector.tensor_tensor(
                    out=v[:, :, :, 0:3],
                    in0=v[:, :, :, 0:3],
                    in1=v[:, :, :, 0:3],
                    op=mybir.AluOpType.mult,
                )
                vector.drain().then_inc(sq_sem, 1)

    return nc


def kernel(x: np.ndarray, prelu_a: np.ndarray, trace: bool = False):
    nc = _build(prelu_a)
    x = np.ascontiguousarray(x, dtype=np.float32)
    in_maps = [
        {"x": x[c * SHARD_ROWS : (c + 1) * SHARD_ROWS]} for c in range(N_CORES)
    ]
    res = run_bass_kernel_spmd(nc, in_maps, list(range(N_CORES)), trace=trace)
    out = np.concatenate([res.results[c]["y"] for c in range(N_CORES)], axis=0)
    if trace:
        return out, res
    return out
